# revision 19
# baseline (speedup 1.0000x reference)
"""Trainium2 Bass kernel for nn_LookAtMappingNetwork (gnn_message_passing).

Strategy
--------
The module's output only reads the final node features at rows R = {i*250 :
i in 0..63} (``ws = x[::250]``).  Working backwards through the two message
-passing processors, only a small data-dependent subset of edges/nodes can
influence those rows, for ANY edge_index:

    E1 = edges with dst in R          (~6 per graph)   -> proc-1 edge MLP
    S  = R  ∪  src[E1]                (~65 per core)   -> rows where x1 needed
    E0 = edges with dst in S          (~375 per core)  -> proc-0 edge MLP

Segment-mean counts stay exact because E0/E1 contain ALL edges landing on
S/R.  Everything else the reference computes is dead code.  Each of the 8
cores handles 8 output rows (its R_c) fully independently; weights are
replicated and streamed from HBM through a small rotating SBUF pool.  All
floating-point math runs on device; the host only does integer index-set
construction (sharding/marshalling).

Layout: proc-0 edge layer 1 runs feature-major (z-contributions arrive via
64-wide selection matmuls, look-at contributions via tiny K=3 matmuls); all
later layers run token-major (tokens<=128 on partitions, 512 output
features on the free axis) with the layer bias injected as an extra K=1
matmul row, so every heavy matmul has a 512-wide moving operand — which is
what lets float32r run the PE at full rate.  leaky_relu(0.2) is composed
from Identity+Relu activations (the HW Lrelu LUT bakes alpha=0.01).
"""

import math

import numpy as np

import concourse.bacc as bacc
import concourse.bass as bass
import concourse.mybir as mybir
import concourse.tile as tile
from concourse.bass import IndirectOffsetOnAxis
from concourse.bass_utils import run_bass_kernel_spmd
from concourse.masks import make_identity

f32 = mybir.dt.float32
fr = mybir.dt.float32r
i32 = mybir.dt.int32
AF = mybir.ActivationFunctionType
OP = mybir.AluOpType

NV = 250
B = 64
D = 512
LR = 0.01
SQ2 = math.sqrt(2.0)
N_CORES = 8
R_PER = B // N_CORES  # output rows per core

CAP_E0 = 512
CAP_S = 128
CAP_E1 = 128

G_E00 = LR / math.sqrt(1034.0)
G_E01 = LR / math.sqrt(512.0)
G_N00 = LR / math.sqrt(1030.0)
G_N01 = LR / math.sqrt(512.0)
G_E10 = LR / math.sqrt(1536.0)
G_E11 = LR / math.sqrt(512.0)
G_N10 = LR / math.sqrt(1024.0)
G_N11 = LR / math.sqrt(512.0)

# agg0 feature splits for the proc-0 node MLP:  [la_mean(3) | ef0_mean(512)]
AGG0_SPLITS = [(0, 3), (3, 131), (131, 259), (259, 387), (387, 515)]


def _build_program():
    """Emit the per-core Bass program (SPMD across 8 cores)."""
    nc = bacc.Bacc("TRN2", target_bir_lowering=False, debug=False,
                   enable_asserts=False, num_devices=N_CORES)

    def din(name, shape, dtype=fr):
        return nc.dram_tensor(name, shape, dtype, kind="ExternalInput")

    z_d = din("z", [B, D])
    la_d = din("lookats", [16000, 3])
    w0e0_zsrc = din("w0e0_zsrc", [512, 512])
    w0e0_zdst = din("w0e0_zdst", [512, 512])
    w0e0_laA = din("w0e0_laA", [3, 512])
    w0e0_laB = din("w0e0_laB", [3, 512])
    w0e0_rel = din("w0e0_rel", [3, 512])
    w0e0_wd = din("w0e0_wd", [1, 512])
    w0e1 = din("w0e1", [512, 512])
    w0n0_z = din("w0n0_z", [512, 512])
    w0n0_la = din("w0n0_la", [3, 512])
    w0n0_agg = din("w0n0_agg", [515, 512])
    w0n1 = din("w0n1", [512, 512])
    w1e0 = din("w1e0", [1536, 512])
    w1e1 = din("w1e1", [512, 512])
    w1n0 = din("w1n0", [1024, 512])
    w1n1 = din("w1n1", [512, 512])
    biases = {k: din("b_" + k, [512]) for k in
              ["e00", "e01", "n00", "n01", "e10", "e11", "n10", "n11"]}
    e0_src_d = din("e0_src", [CAP_E0, 1], i32)
    e0_dst_d = din("e0_dst", [CAP_E0, 1], i32)
    e0_srcmod_d = din("e0_srcmod", [64, CAP_E0])
    e0_dstmod_d = din("e0_dstmod", [64, CAP_E0])
    e0_sigma_d = din("e0_sigma", [CAP_E0], f32)
    s_node_d = din("s_node", [CAP_S, 1], i32)
    s_mod_d = din("s_mod", [64, CAP_S])
    e1_pos_d = din("e1_pos", [128, CAP_E1])
    e1_srcslot_d = din("e1_srcslot", [128, CAP_E1])
    e1_dstslot_d = din("e1_dstslot", [128, CAP_E1])
    e1_sigma_d = din("e1_sigma", [CAP_E1], f32)

    out_d = nc.dram_tensor("out", [R_PER, 14, D], fr, kind="ExternalOutput")

    NT0 = CAP_E0 // 128  # e-tiles in proc-0 edge set
    k4 = [(0, 128), (128, 256), (256, 384), (384, 512)]

    with tile.TileContext(nc) as tc, \
            tc.tile_pool(name="w", bufs=1) as wp, \
            tc.tile_pool(name="wk", bufs=8) as wk, \
            tc.tile_pool(name="tmp", bufs=8) as tp, \
            tc.tile_pool(name="psb", bufs=4, space="PSUM") as psb, \
            tc.tile_pool(name="pss", bufs=4, space="PSUM") as pss:

        # ---------------- constants ----------------
        ident_f = wp.tile([128, 128], f32, name="ident_f")
        make_identity(nc, ident_f[:])
        ident = wp.tile([128, 128], fr, name="ident")
        nc.vector.tensor_copy(ident[:], ident_f[:])
        ones_f32 = wp.tile([128, 1], f32, name="ones_f32")
        nc.gpsimd.memset(ones_f32[:], 1.0)
        iota_free = wp.tile([128, 128], f32, name="iota_free")
        nc.gpsimd.iota(iota_free[:], pattern=[[1, 128]], base=0,
                       channel_multiplier=0, allow_small_or_imprecise_dtypes=True)
        iota_part = []
        for t in range(NT0):
            it = wp.tile([128, 1], f32, name=f"iota_part{t}")
            nc.gpsimd.iota(it[:], pattern=[[1, 1]], base=128 * t,
                           channel_multiplier=1,
                           allow_small_or_imprecise_dtypes=True)
            iota_part.append(it)
        ones_row = wp.tile([1, 128], fr, name="ones_row")
        nc.vector.tensor_copy(ones_row[:], ones_f32[:1, :1].to_broadcast([1, 128]))

        _uid = [0]

        def uid():
            _uid[0] += 1
            return _uid[0]

        def sb(shape, name):
            return wp.tile(shape, fr, name=name)

        _dma_rr = [0]

        def wdma(out_ap, in_ap):
            # Alternate the two HWDGE rings (SP and ACT) so weight streaming
            # isn't serialized on one ring.
            eng = nc.sync if _dma_rr[0] % 2 == 0 else nc.scalar
            _dma_rr[0] += 1
            eng.dma_start(out_ap, in_ap)

        def wtile(dram_t, a, b_):
            t = wk.tile([b_ - a, 512], fr, name=f"wt{uid()}", tag="wk")
            wdma(t[:], dram_t[a:b_, :])
            return t

        def wtile2(dram_t, a):
            """Load rows [a, a+256) as one DMA -> two K-tile views."""
            t = wk.tile([128, 2, 512], fr, name=f"wt{uid()}", tag="wk2")
            wdma(t[:], dram_t[a:a + 256, :].rearrange("(t p) d -> p t d", p=128))
            return [t[:, 0, :], t[:, 1, :]]

        def wtiles_for(dram_t, rows):
            """K-tiles for row ranges; pairs of adjacent 128-rows share a DMA."""
            tiles = []
            i = 0
            while i < len(rows):
                a, b_ = rows[i]
                if (b_ - a == 128 and i + 1 < len(rows)
                        and rows[i + 1] == (b_, b_ + 128)):
                    tiles.extend(wtile2(dram_t, a))
                    i += 2
                else:
                    tiles.append(wtile(dram_t, a, b_)[:])
                    i += 1
            return tiles

        def copyT(src_ap, p, f, dst_ap):
            """PE transpose src [p, f] -> existing sbuf dst_ap [f, p]."""
            ps = pss.tile([f, p], fr, name=f"psT{uid()}", tag="pssm")
            nc.tensor.transpose(ps[:], src_ap, ident[:p, :p])
            nc.vector.tensor_copy(dst_ap, ps[:])

        def peT(src_ap, p, f, name):
            dst = sb([f, p], name)
            copyT(src_ap, p, f, dst[:])
            return dst

        def brow(key, gain):
            """Bias as a K=1 matmul row: (LR/gain) * b, shape [1, 512]."""
            raw = tp.tile([1, 512], fr, name=f"braw{uid()}", tag="ya")
            nc.sync.dma_start(raw[:], biases[key][None, :])
            t = wp.tile([1, 512], fr, name=f"brow_{key}")
            nc.vector.tensor_scalar_mul(t[:], raw[:], LR / gain)
            return t

        def lrelu_tok(psum_ap, gain, out_ap, p, n):
            """out = sqrt2*leaky_relu(gain*acc, 0.2); bias already in acc."""
            ya = tp.tile([p, n], fr, name=f"ya{uid()}", tag="ya")
            nc.scalar.activation(ya[:], psum_ap, AF.Identity,
                                 bias=0.0, scale=0.2 * SQ2 * gain)
            nc.scalar.activation(out_ap, psum_ap, AF.Relu,
                                 bias=0.0, scale=0.8 * SQ2 * gain)
            nc.vector.tensor_add(out_ap, out_ap, ya[:])

        def tok_layer(lhsT_aps, wspec, brow_t, gain, out_ap, p):
            """Token-major FC layer: out[p tokens, 512] = lrelu(in @ W^T + b).

            lhsT_aps: feature-major input K-tiles [k_i, p tokens].
            wspec: matching (dram, row_a, row_b) K-tiles of W^T [K, 512].
            """
            ps = psb.tile([p, 512], f32, name=f"psL{uid()}", tag="psbig")
            wts = wtiles_for(wspec[0][0], [(a, b_) for _, a, b_ in wspec]) \
                if all(w[0] is wspec[0][0] for w in wspec) else None
            for k, ((dt_, a, b_), lh) in enumerate(zip(wspec, lhsT_aps)):
                wt = wts[k] if wts is not None else wtile(dt_, a, b_)[:]
                nc.tensor.matmul(ps[:], lh, wt, start=(k == 0), stop=False)
            nc.tensor.matmul(ps[:], ones_row[:, :p], brow_t[:],
                             start=False, stop=True)
            lrelu_tok(ps[:], gain, out_ap, p, 512)
            return ps

        # ---------------- z normalization ----------------
        zt = tp.tile([64, 512], fr, name="zt", tag="ya")
        nc.sync.dma_start(zt[:], z_d[:, :])
        zsq = tp.tile([64, 512], fr, name="zsq", tag="rr")
        nc.vector.tensor_tensor(zsq[:], zt[:], zt[:], op=OP.mult)
        zss = wp.tile([64, 1], f32, name="zss")
        nc.vector.tensor_reduce(zss[:], zsq[:], axis=mybir.AxisListType.X, op=OP.add)
        nc.vector.tensor_scalar(zss[:], zss[:], 1.0 / 512.0, 1e-8, OP.mult, OP.add)
        zsr = wp.tile([64, 1], f32, name="zsr")
        nc.scalar.sqrt(zsr[:], zss[:])
        zrin = wp.tile([64, 1], f32, name="zrin")
        nc.vector.reciprocal(zrin[:], zsr[:])
        znt = sb([64, 512], "znt")          # zn, token-major [64 z, 512 f]
        nc.vector.tensor_scalar_mul(znt[:], zt[:], zrin[:, :1])

        znT = []                            # zn^T feature-major, 4x [128, 64]
        for k in range(4):
            znT.append(peT(znt[:64, 128 * k:128 * (k + 1)], 64, 128, f"znT{k}"))

        # zterm_A/B [64 z, 512 dout], token-major (no activation, no bias)
        def zterm(dram_t, name):
            ps = psb.tile([64, 512], f32, name=f"ps_{name}", tag="psbig")
            wts = wtiles_for(dram_t, k4)
            for k in range(4):
                nc.tensor.matmul(ps[:], znT[k][:], wts[k],
                                 start=(k == 0), stop=(k == 3))
            t = sb([64, 512], name)
            nc.vector.tensor_copy(t[:], ps[:])
            return t

        ztermA = zterm(w0e0_zsrc, "ztermA")
        ztermB = zterm(w0e0_zdst, "ztermB")

        # edge-encoder look-at weight combos (rel folds into src/dst parts)
        laA = wtile(w0e0_laA, 0, 3)
        laB = wtile(w0e0_laB, 0, 3)
        rel = wtile(w0e0_rel, 0, 3)
        wd = wtile(w0e0_wd, 0, 1)
        laSrcW = sb([3, 512], "laSrcW")     # laA - rel
        nc.vector.tensor_tensor(laSrcW[:], laA[:], rel[:], op=OP.subtract)
        laDstW = sb([3, 512], "laDstW")     # laB + rel
        nc.vector.tensor_tensor(laDstW[:], laB[:], rel[:], op=OP.add)
        wdW = sb([1, 512], "wdW")           # dist column of the edge encoder
        nc.vector.tensor_copy(wdW[:], wd[:])

        # ---------------- proc-0 edge gathers ----------------
        la_src, la_dst, dist, sigma = [], [], [], []
        for t in range(NT0):
            ixs = wp.tile([128, 1], i32, name=f"ixs{t}")
            nc.sync.dma_start(ixs[:], e0_src_d[128 * t:128 * (t + 1), :])
            ixd = wp.tile([128, 1], i32, name=f"ixd{t}")
            nc.sync.dma_start(ixd[:], e0_dst_d[128 * t:128 * (t + 1), :])
            ls = sb([128, 3], f"lasrc{t}")
            nc.gpsimd.indirect_dma_start(
                out=ls[:], out_offset=None, in_=la_d[:],
                in_offset=IndirectOffsetOnAxis(ap=ixs[:, :1], axis=0))
            ld = sb([128, 3], f"ladst{t}")
            nc.gpsimd.indirect_dma_start(
                out=ld[:], out_offset=None, in_=la_d[:],
                in_offset=IndirectOffsetOnAxis(ap=ixd[:, :1], axis=0))
            la_src.append(ls)
            la_dst.append(ld)
            dd = tp.tile([128, 3], fr, name=f"dd{t}", tag="ya")
            nc.vector.tensor_tensor(dd[:], ld[:], ls[:], op=OP.subtract)
            nc.vector.tensor_tensor(dd[:], dd[:], dd[:], op=OP.mult)
            ds = tp.tile([128, 1], f32, name=f"ds{t}", tag="rr")
            nc.vector.tensor_reduce(ds[:], dd[:], axis=mybir.AxisListType.X,
                                    op=OP.add)
            dt_ = sb([128, 1], f"dist{t}")
            nc.scalar.sqrt(dt_[:], ds[:])
            dist.append(dt_)
            sg = wp.tile([128, 1], f32, name=f"sigma{t}")
            nc.sync.dma_start(sg[:], e0_sigma_d[128 * t:128 * (t + 1), None])
            sigma.append(sg)

        # feature-major rhs tiles for the la terms
        laSrcT = sb([3, CAP_E0], "laSrcT")
        laDstT = sb([3, CAP_E0], "laDstT")
        distT = sb([1, CAP_E0], "distT")
        for t in range(NT0):
            copyT(la_src[t][:], 128, 3, laSrcT[:, 128 * t:128 * (t + 1)])
            copyT(la_dst[t][:], 128, 3, laDstT[:, 128 * t:128 * (t + 1)])
            copyT(dist[t][:], 128, 1, distT[:, 128 * t:128 * (t + 1)])

        # z-index selection matrices [64, E0]
        sel0s = sb([64, CAP_E0], "sel0s")
        nc.sync.dma_start(sel0s[:], e0_srcmod_d[:, :])
        sel0d = sb([64, CAP_E0], "sel0d")
        nc.sync.dma_start(sel0d[:], e0_dstmod_d[:, :])
        nc.vector.tensor_scalar(sel0s[:], sel0s[:], iota_part[0][:64, :1], None,
                                OP.is_equal)
        nc.vector.tensor_scalar(sel0d[:], sel0d[:], iota_part[0][:64, :1], None,
                                OP.is_equal)

        # ------------- proc-0 edge MLP layer 1 (feature-major) ------------
        # h0 chunks [128 dout, E0]; bias via per-partition AP on the ACT.
        b_e00_1 = wp.tile([128, 4], fr, name="b_e00_1")
        b_e00_2 = wp.tile([128, 4], fr, name="b_e00_2")
        braw00 = tp.tile([128, 4], fr, name="braw00", tag="ya")
        nc.sync.dma_start(braw00[:], biases["e00"][:].rearrange("(c p) -> p c", p=128))
        nc.vector.tensor_scalar_mul(b_e00_1[:], braw00[:], 0.2 * SQ2 * LR)
        nc.vector.tensor_scalar_mul(b_e00_2[:], braw00[:], 0.8 * SQ2 * LR)

        h0 = []
        for c in range(4):
            cs = slice(128 * c, 128 * (c + 1))
            ps = psb.tile([128, CAP_E0], f32, name=f"ps_efp{c}", tag="psbig")
            nc.tensor.matmul(ps[:], ztermA[:64, cs], sel0s[:], start=True, stop=False)
            nc.tensor.matmul(ps[:], ztermB[:64, cs], sel0d[:], start=False, stop=False)
            nc.tensor.matmul(ps[:], laSrcW[:, cs], laSrcT[:], start=False, stop=False)
            nc.tensor.matmul(ps[:], laDstW[:, cs], laDstT[:], start=False, stop=False)
            nc.tensor.matmul(ps[:], wdW[:, cs], distT[:], start=False, stop=True)
            o = sb([128, CAP_E0], f"h0_{c}")
            ya = tp.tile([128, CAP_E0], fr, name=f"ya0{c}", tag="ya")
            nc.scalar.activation(ya[:], ps[:], AF.Identity,
                                 bias=b_e00_1[:, c:c + 1], scale=0.2 * SQ2 * G_E00)
            nc.scalar.activation(o[:], ps[:], AF.Relu,
                                 bias=b_e00_2[:, c:c + 1], scale=0.8 * SQ2 * G_E00)
            nc.vector.tensor_add(o[:], o[:], ya[:])
            h0.append(o)

        # ------------- proc-0 edge MLP layer 2 (token-major) --------------
        # ef0 written straight into msg tiles: [la_dst(3) | ef0(512) | 1]
        brow_e01 = brow("e01", G_E01)
        w0e1t = wtiles_for(w0e1, k4)
        msg = []
        for t in range(NT0):
            m = sb([128, 516], f"msg{t}")
            nc.vector.tensor_copy(m[:, 0:3], la_dst[t][:])
            nc.vector.tensor_copy(m[:, 515:516], ones_f32[:, :1])
            es = slice(128 * t, 128 * (t + 1))
            ps = psb.tile([128, 512], f32, name=f"ps_ef0{t}", tag="psbig")
            for k in range(4):
                nc.tensor.matmul(ps[:], h0[k][:, es], w0e1t[k],
                                 start=(k == 0), stop=False)
            nc.tensor.matmul(ps[:], ones_row[:, :128], brow_e01[:],
                             start=False, stop=True)
            lrelu_tok(ps[:], G_E01, m[:, 3:515], 128, 512)
            msg.append(m)

        # ---------------- aggregation onto S ----------------
        G0 = []
        for t in range(NT0):
            g = sb([128, 128], f"G0_{t}")
            nc.vector.tensor_scalar(g[:], iota_free[:], sigma[t][:, :1], None,
                                    OP.is_equal)
            G0.append(g)

        ps_a = psb.tile([128, 512], f32, name="ps_agg0a", tag="psbig")
        ps_b = pss.tile([128, 4], f32, name="ps_agg0b", tag="pssm")
        for t in range(NT0):
            nc.tensor.matmul(ps_a[:], G0[t][:], msg[t][:, 0:512],
                             start=(t == 0), stop=(t == NT0 - 1))
            nc.tensor.matmul(ps_b[:], G0[t][:], msg[t][:, 512:516],
                             start=(t == 0), stop=(t == NT0 - 1))
        cnt = wp.tile([128, 1], f32, name="cnt")
        nc.vector.tensor_scalar(cnt[:], ps_b[:, 3:4], 1.0, None, OP.max)
        rin = wp.tile([128, 1], f32, name="rin")
        nc.vector.reciprocal(rin[:], cnt[:])
        # msg feature order is [la(3) | ef(512)], so cols 0:512 of ps_a plus
        # cols 0:3 of ps_b form the contiguous 515-wide [la_mean | ef_mean].
        aggtok = sb([128, 515], "aggtok")   # [S slot, (la_mean|ef_mean)]
        nc.vector.tensor_scalar_mul(aggtok[:, 0:512], ps_a[:, 0:512], rin[:, :1])
        nc.vector.tensor_scalar_mul(aggtok[:, 512:515], ps_b[:, 0:3], rin[:, :1])
        aggT = []
        for j, (a, b_) in enumerate(AGG0_SPLITS):
            aggT.append(peT(aggtok[:, a:b_], 128, b_ - a, f"aggT{j}"))

        # ---------------- node MLP 0 -> x1 (token-major, S slots) ---------
        selS = sb([64, CAP_S], "selS")
        nc.sync.dma_start(selS[:], s_mod_d[:, :])
        nc.vector.tensor_scalar(selS[:], selS[:], iota_part[0][:64, :1], None,
                                OP.is_equal)
        zgS = []
        for c in range(4):
            ps = pss.tile([128, CAP_S], f32, name=f"ps_zg{c}", tag="pssm")
            nc.tensor.matmul(ps[:], znt[:64, 128 * c:128 * (c + 1)], selS[:],
                             start=True, stop=True)
            t_ = sb([128, CAP_S], f"zgS{c}")
            nc.vector.tensor_copy(t_[:], ps[:])
            zgS.append(t_)
        s_ix = wp.tile([CAP_S, 1], i32, name="s_ix")
        nc.sync.dma_start(s_ix[:], s_node_d[:, :])
        laS = sb([CAP_S, 3], "laS")
        nc.gpsimd.indirect_dma_start(
            out=laS[:], out_offset=None, in_=la_d[:],
            in_offset=IndirectOffsetOnAxis(ap=s_ix[:, :1], axis=0))
        laST = peT(laS[:], CAP_S, 3, "laST")

        hn_tok = sb([CAP_S, 512], "hn_tok")
        tok_layer(
            [zgS[k][:] for k in range(4)] + [laST[:]] +
            [aggT[j][:] for j in range(5)],
            [(w0n0_z, a, b_) for a, b_ in k4] + [(w0n0_la, 0, 3)] +
            [(w0n0_agg, a, b_) for a, b_ in AGG0_SPLITS],
            brow("n00", G_N00), G_N00, hn_tok[:], CAP_S)

        hnT = []
        for c in range(4):
            hnT.append(peT(hn_tok[:, 128 * c:128 * (c + 1)], CAP_S, 128,
                           f"hnT{c}"))
        x1tok = sb([CAP_S, 512], "x1tok")
        tok_layer([hnT[k][:] for k in range(4)],
                  [(w0n1, a, b_) for a, b_ in k4],
                  brow("n01", G_N01), G_N01, x1tok[:], CAP_S)

        # x1 at the R slots, feature-major [128 f, 8], via identity columns
        x1R = []
        for c in range(4):
            ps = pss.tile([128, R_PER], f32, name=f"ps_x1R{c}", tag="pssm")
            nc.tensor.matmul(ps[:], x1tok[:, 128 * c:128 * (c + 1)],
                             ident[:CAP_S, 0:R_PER], start=True, stop=True)
            t_ = sb([128, R_PER], f"x1R{c}")
            nc.vector.tensor_copy(t_[:], ps[:])
            x1R.append(t_)

        # ---------------- proc-1 edge MLP (token-major, E1) ---------------
        def load_sel(dram_t, name, nt=1):
            raw = sb([128, CAP_E1], f"{name}raw")
            nc.sync.dma_start(raw[:], dram_t[:, :])
            sels = []
            for t in range(nt):
                s_ = raw if nt == 1 else sb([128, CAP_E1], f"{name}{t}")
                nc.vector.tensor_scalar(s_[:], raw[:], iota_part[t][:, :1],
                                        None, OP.is_equal)
                sels.append(s_)
            return sels

        selA = load_sel(e1_srcslot_d, "selA")[0]
        selB = load_sel(e1_dstslot_d, "selB")[0]
        selE = load_sel(e1_pos_d, "selE", nt=NT0)

        def sel_gather(lhsT_fns, sel_tiles, name, n=CAP_E1):
            outs = []
            for c in range(4):
                ps = pss.tile([128, n], f32, name=f"ps_{name}{c}", tag="pssm")
                for t, s_ in enumerate(sel_tiles):
                    nc.tensor.matmul(ps[:], lhsT_fns[t](c), s_[:],
                                     start=(t == 0), stop=(t == len(sel_tiles) - 1))
                o = sb([128, n], f"{name}{c}")
                nc.vector.tensor_copy(o[:], ps[:])
                outs.append(o)
            return outs

        x1gA = sel_gather([lambda c: x1tok[:, 128 * c:128 * (c + 1)]], [selA], "x1gA")
        x1gB = sel_gather([lambda c: x1tok[:, 128 * c:128 * (c + 1)]], [selB], "x1gB")
        ef0g = sel_gather(
            [(lambda t: (lambda c: msg[t][:, 3 + 128 * c:3 + 128 * (c + 1)]))(t)
             for t in range(NT0)], selE, "ef0g")

        h1tok = sb([CAP_E1, 512], "h1tok")
        tok_layer([r[:] for r in (x1gA + x1gB + ef0g)],
                  [(w1e0, 128 * i, 128 * (i + 1)) for i in range(12)],
                  brow("e10", G_E10), G_E10, h1tok[:], CAP_E1)

        h1T = []
        for c in range(4):
            h1T.append(peT(h1tok[:, 128 * c:128 * (c + 1)], CAP_E1, 128,
                           f"h1T{c}"))
        # ef1 written straight into msg1 cols 0:512 (token-major already)
        msg1 = sb([CAP_E1, 514], "msg1")
        nc.vector.tensor_copy(msg1[:, 512:514],
                              ones_f32[:, 0:1].to_broadcast([128, 2]))
        tok_layer([h1T[k][:] for k in range(4)],
                  [(w1e1, a, b_) for a, b_ in k4],
                  brow("e11", G_E11), G_E11, msg1[:, 0:512], CAP_E1)

        # ---------------- aggregation onto R (8 rows) ---------------------
        e1sig = wp.tile([CAP_E1, 1], f32, name="e1sig")
        nc.sync.dma_start(e1sig[:], e1_sigma_d[:, None])
        G1 = sb([CAP_E1, R_PER], "G1")
        nc.vector.tensor_scalar(G1[:], iota_free[:, 0:R_PER], e1sig[:, :1], None,
                                OP.is_equal)
        ps1 = psb.tile([R_PER, 512], f32, name="ps_agg1", tag="psbig")
        nc.tensor.matmul(ps1[:], G1[:], msg1[:, 0:512], start=True, stop=True)
        ps2 = pss.tile([R_PER, 2], f32, name="ps_agg1b", tag="pssm")
        nc.tensor.matmul(ps2[:], G1[:], msg1[:, 512:514], start=True, stop=True)
        cnt1 = wp.tile([R_PER, 1], f32, name="cnt1")
        nc.vector.tensor_scalar(cnt1[:], ps2[:, 0:1], 1.0, None, OP.max)
        rin1 = wp.tile([R_PER, 1], f32, name="rin1")
        nc.vector.reciprocal(rin1[:], cnt1[:])
        agg1tok = sb([R_PER, 512], "agg1tok")
        nc.vector.tensor_scalar_mul(agg1tok[:], ps1[:], rin1[:, :1])
        agg1T = []
        for c in range(4):
            agg1T.append(peT(agg1tok[:R_PER, 128 * c:128 * (c + 1)], R_PER, 128,
                             f"agg1T{c}"))

        # ---------------- final node MLP (token-major, 8 rows) ------------
        hftok = sb([R_PER, 512], "hftok")
        tok_layer([x1R[k][:] for k in range(4)] + [agg1T[k][:] for k in range(4)],
                  [(w1n0, 128 * i, 128 * (i + 1)) for i in range(8)],
                  brow("n10", G_N10), G_N10, hftok[:], R_PER)
        hfT = []
        for c in range(4):
            hfT.append(peT(hftok[:R_PER, 128 * c:128 * (c + 1)], R_PER, 128,
                           f"hfT{c}"))
        wstok = sb([R_PER, 512], "wstok")
        tok_layer([hfT[k][:] for k in range(4)],
                  [(w1n1, a, b_) for a, b_ in k4],
                  brow("n11", G_N11), G_N11, wstok[:], R_PER)

        for j in range(14):
            nc.sync.dma_start(out_d[:, j, :], wstok[:])

    nc.finalize()
    return nc


_PROG_CACHE = {}


def _get_program():
    key = (CAP_E0, CAP_S, CAP_E1)
    if key not in _PROG_CACHE:
        _PROG_CACHE[key] = _build_program()
    return _PROG_CACHE[key]


def _pad(a, n, fill, dtype):
    out = np.full((n,), fill, dtype=dtype)
    out[:len(a)] = a.astype(dtype)
    return out


def _bcast(row, p):
    return np.ascontiguousarray(np.broadcast_to(row[None, :].astype(np.float32),
                                                (p, row.shape[0])))


def _core_inputs(src, dst, c):
    Rc = (np.arange(R_PER, dtype=np.int64) + c * R_PER) * NV
    E1 = np.nonzero(np.isin(dst, Rc))[0]
    others = np.setdiff1d(np.unique(src[E1]), Rc)
    S = np.concatenate([Rc, others])
    assert len(E1) <= CAP_E1 and len(S) <= CAP_S, (len(E1), len(S))
    slot = np.full(16000, -1, np.int64)
    slot[S] = np.arange(len(S))
    E0 = np.nonzero(slot[dst] >= 0)[0]
    assert len(E0) <= CAP_E0, len(E0)
    pos = np.full(src.shape[0], -1, np.int64)
    pos[E0] = np.arange(len(E0))
    e0s, e0d = src[E0], dst[E0]
    e1s, e1d = src[E1], dst[E1]
    return {
        "e0_src": _pad(e0s, CAP_E0, 0, np.int32)[:, None],
        "e0_dst": _pad(e0d, CAP_E0, 0, np.int32)[:, None],
        "e0_srcmod": _bcast(_pad(e0s % B, CAP_E0, 0, np.float32), 64),
        "e0_dstmod": _bcast(_pad(e0d % B, CAP_E0, 0, np.float32), 64),
        "e0_sigma": _pad(slot[e0d], CAP_E0, -1, np.float32),
        "s_node": _pad(S, CAP_S, 0, np.int32)[:, None],
        "s_mod": _bcast(_pad(S % B, CAP_S, 0, np.float32), 64),
        "e1_pos": _bcast(_pad(pos[E1], CAP_E1, -1, np.float32), 128),
        "e1_srcslot": _bcast(_pad(slot[e1s], CAP_E1, -1, np.float32), 128),
        "e1_dstslot": _bcast(_pad(slot[e1d], CAP_E1, -1, np.float32), 128),
        "e1_sigma": _pad(slot[e1d], CAP_E1, -1, np.float32),
    }


def _host_inputs(inputs):
    z = np.ascontiguousarray(np.asarray(inputs["z"], np.float32))
    la = np.ascontiguousarray(np.asarray(inputs["look_ats"], np.float32))

    def T(a):
        return np.ascontiguousarray(np.asarray(a, np.float32).T)

    w0e0T = T(inputs["p0_ew0"])
    w0n0T = T(inputs["p0_nw0"])
    return {
        "z": z, "lookats": la,
        "w0e0_zsrc": np.ascontiguousarray(w0e0T[0:512]),
        "w0e0_zdst": np.ascontiguousarray(w0e0T[515:1027]),
        "w0e0_laA": np.ascontiguousarray(w0e0T[512:515]),
        "w0e0_laB": np.ascontiguousarray(w0e0T[1027:1030]),
        "w0e0_rel": np.ascontiguousarray(w0e0T[1030:1033]),
        "w0e0_wd": np.ascontiguousarray(w0e0T[1033:1034]),
        "w0e1": T(inputs["p0_ew1"]),
        "w0n0_z": np.ascontiguousarray(w0n0T[0:512]),
        "w0n0_la": np.ascontiguousarray(w0n0T[512:515]),
        "w0n0_agg": np.ascontiguousarray(w0n0T[515:1030]),
        "w0n1": T(inputs["p0_nw1"]),
        "w1e0": T(inputs["p1_ew0"]),
        "w1e1": T(inputs["p1_ew1"]),
        "w1n0": T(inputs["p1_nw0"]),
        "w1n1": T(inputs["p1_nw1"]),
        "b_e00": np.asarray(inputs["p0_eb0"], np.float32),
        "b_e01": np.asarray(inputs["p0_eb1"], np.float32),
        "b_n00": np.asarray(inputs["p0_nb0"], np.float32),
        "b_n01": np.asarray(inputs["p0_nb1"], np.float32),
        "b_e10": np.asarray(inputs["p1_eb0"], np.float32),
        "b_e11": np.asarray(inputs["p1_eb1"], np.float32),
        "b_n10": np.asarray(inputs["p1_nb0"], np.float32),
        "b_n11": np.asarray(inputs["p1_nb1"], np.float32),
    }


def make_in_maps(inputs):
    ei = np.asarray(inputs["edge_index"])
    src, dst = ei[0].astype(np.int64), ei[1].astype(np.int64)
    shared = _host_inputs(inputs)
    return [dict(shared, **_core_inputs(src, dst, c)) for c in range(N_CORES)]


def kernel(**inputs):
    nc = _get_program()
    in_maps = make_in_maps(inputs)
    res = run_bass_kernel_spmd(nc, in_maps, core_ids=list(range(N_CORES)))
    out = np.concatenate([res.results[c]["out"] for c in range(N_CORES)], axis=0)
    return out.astype(np.float32)
